# revision 14
# baseline (speedup 1.0000x reference)
"""Trainium2 Bass kernel for AttentionDecoderModel (6-layer transformer decoder).

Sharding: pure data-parallel over batch. B=8 batch elements -> 8 NeuronCores,
one batch element per core, no collectives.

Per-core layout: activations are kept feature-major ([D partitions, T free],
chunked into 128-partition tiles). Matmuls run in bf16 with f32 PSUM
accumulation. LayerNorm statistics are computed with TensorE ones-matmuls
(partition-dim reduction) and broadcast back with K=1 matmuls. Softmax is
computed without max subtraction (scores are O(1) by construction: LN'd
activations through 0.02-scale weights); masks are folded into the score
matmul (cross-attn, via an appended K=1 bias-row matmul) or one binary
multiply (self-attn causal+pad mask).

Host side: embedding gather + positional encoding, mask construction, weight
transposition/folding (LN affine, biases, 1/sqrt(HD)) and bf16 casting.
"""

import math

import numpy as np
import ml_dtypes

import concourse.bass as bass  # noqa: F401
import concourse.mybir as mybir
import concourse.tile as tile
from concourse import bacc
from concourse.bass_utils import run_bass_kernel_spmd

AF = mybir.ActivationFunctionType
ALU = mybir.AluOpType
F32 = mybir.dt.float32
BF16 = mybir.dt.bfloat16
BF16NP = ml_dtypes.bfloat16

B, T, S = 8, 512, 1024
D, AD, H, FF, MD, L, V = 512, 512, 8, 2048, 512, 6, 8000
HD = AD // H            # 64
KC = D // 128           # 4
TC = T // 128           # 4
SC = S // 128           # 8
FC = FF // 128          # 16
VG = 16                 # vocab DMA groups
VWID = V // VG          # 500 cols per group (fits one PSUM bank)
EPS = 1e-5
NEG = -800.0            # masked-score additive bias (applied pre-1/8 scaling... here direct)

# bias column layout in the per-layer [128, 44] bias tensor
QB0, KB0, OB0, CQB0, CKB0, COB0, F2B0, F1B0 = 0, 4, 8, 12, 16, 20, 24, 28
NBIAS = 44

_CACHE = {}


def _build():
    from concourse import hw_specs

    nc = bacc.Bacc(None, target_bir_lowering=False, debug=False)
    # Force Exp into natural_log_exp_and_others so LN's ln/exp-based rsqrt and
    # attention's exp share one ACT table set (avoids per-LN table thrashing).
    tabs = hw_specs.get_activation_tables(nc.m.arch)
    tabs["exp_and_others"].discard(AF.Exp)
    tabs["natural_log"].discard(AF.Ln)

    # ---- DRAM parameters
    h0_ext = nc.declare_dram_parameter("h0", [KC, 128, T], F32, isOutput=False)
    tri_ext = nc.declare_dram_parameter("tri", [128, 128], BF16, isOutput=False)
    padb_ext = nc.declare_dram_parameter("padb", [128, TC], F32, isOutput=False)
    mbias_ext = nc.declare_dram_parameter("membias", [1, S], BF16, isOutput=False)
    memT_ext = nc.declare_dram_parameter("memT", [KC, 128, S], BF16, isOutput=False)
    outw_ext = nc.declare_dram_parameter("outw", [KC, 128, V], BF16, isOutput=False)
    out_ext = nc.declare_dram_parameter("out", [T, V], F32, isOutput=True)
    wself_ext, wcross_ext, wf1_ext, wf2_ext, bias_ext = [], [], [], [], []
    for l in range(L):
        wself_ext.append(nc.declare_dram_parameter(
            f"L{l}_wself", [4, KC, 128, 512], BF16, isOutput=False))
        wcross_ext.append(nc.declare_dram_parameter(
            f"L{l}_wcross", [4, KC, 128, 512], BF16, isOutput=False))
        wf1_ext.append(nc.declare_dram_parameter(
            f"L{l}_wf1", [KC, 128, FF], BF16, isOutput=False))
        wf2_ext.append(nc.declare_dram_parameter(
            f"L{l}_wf2", [FC, 128, 512], BF16, isOutput=False))
        bias_ext.append(nc.declare_dram_parameter(
            f"L{l}_bias", [128, NBIAS], F32, isOutput=False))

    with tile.TileContext(nc) as tc:
        with (
            tc.tile_pool(name="sb", bufs=1) as sb,
            tc.tile_pool(name="ps", bufs=2, space="PSUM") as pp,
        ):
            # ---- constants
            ones128_bf = sb.tile([128, 1], BF16, tag="c1")
            nc.any.memset(ones128_bf[:], 1.0)
            ones128_f = sb.tile([128, 1], F32, tag="c2")
            nc.any.memset(ones128_f[:], 1.0)
            onesrow_f = sb.tile([1, 128], F32, tag="c3")
            nc.any.memset(onesrow_f[:], 1.0)
            onesrow_bf = sb.tile([1, T], BF16, tag="c4")
            nc.any.memset(onesrow_bf[:], 1.0)
            eps_c = sb.tile([1, 1], F32, tag="c6")
            nc.any.memset(eps_c[:], EPS)

            # ---- resident inputs
            tri = sb.tile([128, 128], BF16, tag="tri")
            nc.sync.dma_start(tri[:], tri_ext[:])
            padb = sb.tile([128, TC], F32, tag="padb")
            nc.sync.dma_start(padb[:], padb_ext[:])
            mbias = sb.tile([1, S], BF16, tag="mbias")
            nc.sync.dma_start(mbias[:], mbias_ext[:])
            memT = sb.tile([128, KC, S], BF16, tag="memT")
            for c in range(KC):
                nc.sync.dma_start(memT[:, c, :], memT_ext[c])

            h = sb.tile([128, KC, T], F32, tag="h", bufs=2)
            for c in range(KC):
                nc.sync.dma_start(h[:, c, :], h0_ext[c])

            def ln(h_in):
                """feature-major LayerNorm (no affine): bf16 [128, KC, T]."""
                sq = sb.tile([128, KC, T], F32, tag="sq")
                nc.vector.tensor_mul(sq[:], h_in[:], h_in[:])
                s_ps = pp.tile([1, T], F32, tag="zr")
                for c in range(KC):
                    nc.tensor.matmul(s_ps[:], ones128_f[:], h_in[:, c, :],
                                     start=(c == 0), stop=(c == KC - 1))
                sq_ps = pp.tile([1, T], F32, tag="zr")
                for c in range(KC):
                    nc.tensor.matmul(sq_ps[:], ones128_f[:], sq[:, c, :],
                                     start=(c == 0), stop=(c == KC - 1))
                ms = sb.tile([1, T], F32, tag="ms")
                nc.vector.tensor_scalar_mul(ms[:], s_ps[:], 1.0 / D)
                m2 = sb.tile([1, T], F32, tag="m2")
                nc.vector.tensor_mul(m2[:], ms[:], ms[:])
                var = sb.tile([1, T], F32, tag="var")
                nc.vector.scalar_tensor_tensor(var[:], sq_ps[:], 1.0 / D, m2[:],
                                               ALU.mult, ALU.subtract)
                lnv = sb.tile([1, T], F32, tag="lnv")
                nc.scalar.activation(lnv[:], var[:], AF.Ln, bias=eps_c[:], scale=1.0)
                rstd = sb.tile([1, T], F32, tag="rstd")
                nc.scalar.activation(rstd[:], lnv[:], AF.Exp, bias=0.0, scale=-0.5)
                mean_ps = pp.tile([128, 512], F32, tag="mm")
                nc.tensor.matmul(mean_ps[:], onesrow_f[:], ms[:], start=True, stop=True)
                rstd_ps = pp.tile([128, 512], F32, tag="mm")
                nc.tensor.matmul(rstd_ps[:], onesrow_f[:], rstd[:], start=True, stop=True)
                normt = sb.tile([128, KC, T], BF16, tag="normt")
                for c in range(KC):
                    t1 = sb.tile([128, T], F32, tag="t1", bufs=2)
                    nc.vector.tensor_sub(t1[:], h_in[:, c, :], mean_ps[:])
                    nc.vector.tensor_mul(normt[:, c, :], t1[:], rstd_ps[:])
                return normt

            def project_fm(w_sb, wi, rhs_t, bias_t, c0):
                """feature-major projection -> bf16 [128, KC, T]."""
                outt = sb.tile([128, KC, T], BF16, tag="proj", bufs=2)
                for m in range(KC):
                    ps = pp.tile([128, 512], F32, tag="mm")
                    for kc in range(KC):
                        nc.tensor.matmul(
                            ps[:], w_sb[:, wi, kc, m * 128:(m + 1) * 128],
                            rhs_t[:, kc, :],
                            start=(kc == 0), stop=(kc == KC - 1))
                    nc.vector.tensor_scalar_add(outt[:, m, :], ps[:],
                                                bias_t[:, c0 + m:c0 + m + 1])
                return outt

            def attention(qt, kt, vrow, n_sc, causal, bias_row, tag):
                """qt bf16 [128,KC,T] (pre-scaled 1/8); kt bf16 [128,KC,n_sc*128];
                vrow bf16 [128,n_sc,512]; returns normalized attnT bf16 [128,KC,T].
                causal=True: self-attn — restrict all work to t >= 128*c, pad mask
                via per-partition exp bias, triangle mask on the diagonal block."""
                attnt = sb.tile([128, KC, T], BF16, tag="attnt")
                avps = zbc_ps = None
                for hh in range(H):
                    mc, hr = hh // 2, (hh % 2) * 64
                    p_sb = sb.tile([128, n_sc, T], BF16, tag=tag, bufs=(2 if n_sc == TC else 1))
                    for g in range(n_sc // 2):
                        ps = pp.tile([128, 2, 512], F32, tag="sc")
                        for j in range(2):
                            c = 2 * g + j
                            t0 = 128 * c if causal else 0
                            nc.tensor.matmul(
                                ps[:, j, :T - t0],
                                kt[hr:hr + 64, mc, c * 128:(c + 1) * 128],
                                qt[hr:hr + 64, mc, t0:],
                                start=True, stop=(bias_row is None))
                            if bias_row is not None:
                                nc.tensor.matmul(
                                    ps[:, j, :],
                                    bias_row[0:1, c * 128:(c + 1) * 128],
                                    onesrow_bf[:],
                                    start=False, stop=True)
                            if causal:
                                nc.scalar.activation(
                                    p_sb[:, c, t0:], ps[:, j, :T - t0],
                                    AF.Exp, bias=padb[:, c:c + 1], scale=1.0)
                                nc.vector.tensor_mul(
                                    p_sb[:, c, t0:t0 + 128],
                                    p_sb[:, c, t0:t0 + 128], tri[:])
                        if not causal:
                            nc.scalar.activation(p_sb[:, 2 * g:2 * g + 2, :], ps[:],
                                                 AF.Exp, bias=0.0, scale=1.0)
                    zps = pp.tile([1, T], F32, tag="zr")
                    for c in range(n_sc):
                        t0 = 128 * c if causal else 0
                        nc.tensor.matmul(zps[0:1, t0:], ones128_bf[:],
                                         p_sb[:, c, t0:],
                                         start=(c == 0), stop=(c == n_sc - 1))
                    rz = sb.tile([1, T], F32, tag="rz", bufs=2)
                    nc.vector.reciprocal_approx_fast(rz[:], zps[:])
                    if hh % 2 == 0:
                        avps = pp.tile([128, 512], F32, tag="mm")
                        zbc_ps = pp.tile([128, 512], F32, tag="mm")
                    tp = (0, hr) if hr else None
                    for c in range(n_sc):
                        t0 = 128 * c if causal else 0
                        nc.tensor.matmul(avps[hr:hr + 64, t0:],
                                         vrow[:, c, hh * 64:(hh + 1) * 64],
                                         p_sb[:, c, t0:],
                                         start=(c == 0), stop=(c == n_sc - 1),
                                         tile_position=tp)
                    nc.tensor.matmul(zbc_ps[hr:hr + 64, :], onesrow_f[0:1, 0:64],
                                     rz[:], start=True, stop=True,
                                     tile_position=tp)
                    if hh % 2 == 1:
                        zbc = sb.tile([128, T], F32, tag="zbc")
                        nc.vector.tensor_copy(zbc[:], zbc_ps[:])
                        nc.vector.tensor_mul(attnt[:, mc, :], avps[:], zbc[:])
                return attnt

            def out_proj_residual(w_sb, wi, attnt, h_in, bias_t, c0):
                h_out = sb.tile([128, KC, T], F32, tag="h", bufs=2)
                for m in range(KC):
                    ps = pp.tile([128, 512], F32, tag="mm")
                    for kc in range(KC):
                        nc.tensor.matmul(
                            ps[:], w_sb[:, wi, kc, m * 128:(m + 1) * 128],
                            attnt[:, kc, :],
                            start=(kc == 0), stop=(kc == KC - 1))
                    nc.vector.scalar_tensor_tensor(
                        h_out[:, m, :], ps[:], bias_t[:, c0 + m:c0 + m + 1],
                        h_in[:, m, :], ALU.add, ALU.add)
                return h_out

            # ================= layers =================
            for l in range(L):
                wself = sb.tile([128, 4, KC, 512], BF16, tag="wself")
                for w in range(4):
                    for kc in range(KC):
                        nc.sync.dma_start(wself[:, w, kc, :], wself_ext[l][w, kc])
                wcross = sb.tile([128, 4, KC, 512], BF16, tag="wcross")
                for w in range(4):
                    for kc in range(KC):
                        nc.sync.dma_start(wcross[:, w, kc, :], wcross_ext[l][w, kc])
                wf1 = sb.tile([128, KC, FF], BF16, tag="wf1")
                for kc in range(KC):
                    nc.sync.dma_start(wf1[:, kc, :], wf1_ext[l][kc])
                wf2 = sb.tile([128, FC, 512], BF16, tag="wf2")
                for kc in range(FC):
                    nc.sync.dma_start(wf2[:, kc, :], wf2_ext[l][kc])
                bia = sb.tile([128, NBIAS], F32, tag="bias", bufs=2)
                nc.sync.dma_start(bia[:], bias_ext[l][:])

                # ---- self attention
                n1 = ln(h)
                qt = project_fm(wself, 0, n1, bia, QB0)
                kt = project_fm(wself, 1, n1, bia, KB0)
                vrow = sb.tile([128, TC, 512], BF16, tag="vrow")
                for m in range(TC):
                    ps = pp.tile([128, 512], F32, tag="mm")
                    for kc in range(KC):
                        nc.tensor.matmul(
                            ps[:], n1[:, kc, m * 128:(m + 1) * 128],
                            wself[:, 2, kc, :],
                            start=(kc == 0), stop=(kc == KC - 1))
                    nc.vector.tensor_copy(vrow[:, m, :], ps[:])
                at = attention(qt, kt, vrow, TC, True, None, "pself")
                h = out_proj_residual(wself, 3, at, h, bia, OB0)

                # ---- cross attention
                n2 = ln(h)
                qt2 = project_fm(wcross, 0, n2, bia, CQB0)
                kt2 = sb.tile([128, KC, S], BF16, tag="kt2")
                for mc in range(KC):
                    for half in range(2):
                        ps = pp.tile([128, 512], F32, tag="mm")
                        for kc in range(KC):
                            nc.tensor.matmul(
                                ps[:], wcross[:, 1, kc, mc * 128:(mc + 1) * 128],
                                memT[:, kc, half * 512:(half + 1) * 512],
                                start=(kc == 0), stop=(kc == KC - 1))
                        nc.vector.tensor_scalar_add(
                            kt2[:, mc, half * 512:(half + 1) * 512], ps[:],
                            bia[:, CKB0 + mc:CKB0 + mc + 1])
                vrow2 = sb.tile([128, SC, 512], BF16, tag="vrow2")
                for m in range(SC):
                    ps = pp.tile([128, 512], F32, tag="mm")
                    for kc in range(KC):
                        nc.tensor.matmul(
                            ps[:], memT[:, kc, m * 128:(m + 1) * 128],
                            wcross[:, 2, kc, :],
                            start=(kc == 0), stop=(kc == KC - 1))
                    nc.vector.tensor_copy(vrow2[:, m, :], ps[:])
                at2 = attention(qt2, kt2, vrow2, SC, False, mbias, "pcross")
                h = out_proj_residual(wcross, 3, at2, h, bia, COB0)

                # ---- FFN
                n3 = ln(h)
                ft = sb.tile([128, FC, T], BF16, tag="ft")
                for mf in range(FC):
                    ps = pp.tile([128, 512], F32, tag="mm")
                    for kc in range(KC):
                        nc.tensor.matmul(
                            ps[:], wf1[:, kc, mf * 128:(mf + 1) * 128],
                            n3[:, kc, :],
                            start=(kc == 0), stop=(kc == KC - 1))
                    nc.scalar.activation(ft[:, mf, :], ps[:], AF.Silu,
                                         bias=bia[:, F1B0 + mf:F1B0 + mf + 1],
                                         scale=1.0)
                h_out = sb.tile([128, KC, T], F32, tag="h", bufs=2)
                for m in range(KC):
                    ps = pp.tile([128, 512], F32, tag="mm")
                    for kc in range(FC):
                        nc.tensor.matmul(
                            ps[:], wf2[:, kc, m * 128:(m + 1) * 128],
                            ft[:, kc, :],
                            start=(kc == 0), stop=(kc == FC - 1))
                    nc.vector.scalar_tensor_tensor(
                        h_out[:, m, :], ps[:], bia[:, F2B0 + m:F2B0 + m + 1],
                        h[:, m, :], ALU.add, ALU.add)
                h = h_out

            # ================= output projection =================
            hb = sb.tile([128, KC, T], BF16, tag="normt")
            for c in range(KC):
                nc.vector.tensor_copy(hb[:, c, :], h[:, c, :])
            for g in range(VG):
                wo = sb.tile([128, KC, VWID], BF16, tag="wo", bufs=2)
                for kc in range(KC):
                    nc.sync.dma_start(wo[:, kc, :],
                                      outw_ext[kc, :, g * VWID:(g + 1) * VWID])
                for m in range(TC):
                    ps = pp.tile([128, 512], F32, tag="mm")
                    for kc in range(KC):
                        nc.tensor.matmul(
                            ps[:, :VWID], hb[:, kc, m * 128:(m + 1) * 128],
                            wo[:, kc, :],
                            start=(kc == 0), stop=(kc == KC - 1))
                    lo = sb.tile([128, VWID], F32, tag="lo", bufs=2)
                    if (g + m) % 2 == 0:
                        nc.vector.tensor_copy(lo[:], ps[:, :VWID])
                    else:
                        nc.scalar.copy(lo[:], ps[:, :VWID])
                    nc.sync.dma_start(
                        out_ext[m * 128:(m + 1) * 128, g * VWID:(g + 1) * VWID],
                        lo[:])

    nc.compile()
    return nc


def _pos_encoding(n, d):
    pos = np.arange(n, dtype=np.float32)[:, None]
    div = np.exp(np.arange(0, d, 2, dtype=np.float32) * (-np.log(10000.0) / d))
    pe = np.zeros((n, d), np.float32)
    pe[:, 0::2] = np.sin(pos * div)
    pe[:, 1::2] = np.cos(pos * div)
    return pe


def _tp(w):
    """W [out,in] f32 -> W.T chunked [in/128, 128, out] bf16."""
    w = np.ascontiguousarray(np.asarray(w, np.float32).T)
    return w.reshape(w.shape[0] // 128, 128, w.shape[1]).astype(BF16NP)


def _prep(inputs):
    x = np.asarray(inputs["x"]).astype(np.int64)
    x_lens = np.asarray(inputs["x_lens"]).astype(np.int64)
    memory = np.asarray(inputs["memory"], np.float32)
    memory_lens = np.asarray(inputs["memory_lens"]).astype(np.int64)
    embed = np.asarray(inputs["embed"], np.float32)
    lp = {k: np.asarray(v, np.float32) for k, v in inputs["layer_params"].items()}
    out_w = np.asarray(inputs["out_w"], np.float32)
    out_b = np.asarray(inputs["out_b"], np.float32)

    pe = _pos_encoding(T, D)
    common = {"outw": _tp(out_w)}

    for l in range(L):
        g = {k: v[l] for k, v in lp.items()}
        sc = 1.0 / math.sqrt(HD)
        qw_s = (g["sa_qw"] * g["ln1_w"][None, :]) * sc
        qb_s = (g["sa_qb"] + g["sa_qw"] @ g["ln1_b"]) * sc
        kw_s = g["sa_kw"] * g["ln1_w"][None, :]
        kb_s = g["sa_kb"] + g["sa_kw"] @ g["ln1_b"]
        vw_s = g["sa_vw"] * g["ln1_w"][None, :]
        vb_s = g["sa_vb"] + g["sa_vw"] @ g["ln1_b"]
        ob_s = g["sa_ob"] + g["sa_ow"] @ vb_s
        qw_c = (g["ca_qw"] * g["ln2_w"][None, :]) * sc
        qb_c = (g["ca_qb"] + g["ca_qw"] @ g["ln2_b"]) * sc
        ob_c = g["ca_ob"] + g["ca_ow"] @ g["ca_vb"]
        f1w = g["ff1_w"] * g["ln3_w"][None, :]
        f1b = g["ff1_b"] + g["ff1_w"] @ g["ln3_b"]

        common[f"L{l}_wself"] = np.stack(
            [_tp(qw_s), _tp(kw_s), _tp(vw_s), _tp(g["sa_ow"])])
        common[f"L{l}_wcross"] = np.stack(
            [_tp(qw_c), _tp(g["ca_kw"]), _tp(g["ca_vw"]), _tp(g["ca_ow"])])
        common[f"L{l}_wf1"] = _tp(f1w)
        common[f"L{l}_wf2"] = _tp(g["ff2_w"])
        bias = np.zeros((128, NBIAS), np.float32)
        for m in range(4):
            bias[:, QB0 + m] = qb_s[m * 128:(m + 1) * 128]
            bias[:, KB0 + m] = kb_s[m * 128:(m + 1) * 128]
            bias[:, OB0 + m] = ob_s[m * 128:(m + 1) * 128]
            bias[:, CQB0 + m] = qb_c[m * 128:(m + 1) * 128]
            bias[:, CKB0 + m] = g["ca_kb"][m * 128:(m + 1) * 128]
            bias[:, COB0 + m] = ob_c[m * 128:(m + 1) * 128]
            bias[:, F2B0 + m] = g["ff2_b"][m * 128:(m + 1) * 128]
        for m in range(FC):
            bias[:, F1B0 + m] = f1b[m * 128:(m + 1) * 128]
        common[f"L{l}_bias"] = bias

    in_maps = []
    srange = np.arange(T)
    for b in range(B):
        m = dict(common)
        h0 = embed[x[b]] * np.float32(math.sqrt(D)) + pe        # [T, D]
        m["h0"] = np.ascontiguousarray(h0.T).reshape(KC, 128, T).astype(np.float32)
        loc = np.arange(128)
        m["tri"] = (loc[:, None] <= loc[None, :]).astype(np.float32).astype(BF16NP)
        m["padb"] = np.where(srange.reshape(TC, 128).T < x_lens[b],
                             0.0, np.float32(NEG)).astype(np.float32)
        mb = np.where(np.arange(S) < memory_lens[b], 0.0, np.float32(NEG))
        m["membias"] = mb.reshape(1, S).astype(np.float32).astype(BF16NP)
        m["memT"] = np.ascontiguousarray(memory[b].T).reshape(KC, 128, S).astype(BF16NP)
        in_maps.append(m)
    return in_maps, out_b


def kernel(**inputs):
    if "nc" not in _CACHE:
        _CACHE["nc"] = _build()
    nc = _CACHE["nc"]
    in_maps, out_b = _prep(inputs)
    res = run_bass_kernel_spmd(nc, in_maps, list(range(B)))
    out = np.stack([res.results[i]["out"] for i in range(B)])
    return (out + out_b[None, None, :]).astype(np.float32)
